# revision 33
# baseline (speedup 1.0000x reference)
"""MoE cascaded FFN (top-2, capacity-dispatched) on 8 Trainium2 NeuronCores.

Strategy: hidden-dim (H) sharding — perfectly load-balanced SPMD.
  - Host: gating softmax + top-2 + GShard k-major capacity dispatch
    (pure bookkeeping + gathers). Pack ALL experts' valid rows into one
    global row list sorted by (expert, slot), each expert padded to a
    multiple of 128 rows -> dispT [128, KT, R] (R ~= 16.6k rows vs
    8*2304 = 18.4k padded rows for expert-per-core).
  - Device (SPMD, identical program AND identical schedule on every
    core; only the weight contents differ): core i owns H-dims
    [i*512, (i+1)*512) of EVERY expert.
        hT = relu(W1_slice @ dispT + b1_slice)    (per-expert chunks)
        y_partial = hT.T @ W2_slice               fp16 partial sums
    Both weight slabs (8.4 MB each, bf16) are SBUF-resident, loaded
    once. PE work is identical on all cores regardless of routing
    skew -> no capacity-imbalance waste.
  - Host: sum the 8 partial y's, gather rows back per (token, k),
    weight by normalized gates, add the fc2 bias contribution.
"""

import numpy as np
import ml_dtypes

T, M, H, E, K = 8192, 1024, 4096, 8, 2
CAP = 2560
N_CORES = 8

HL = H // N_CORES          # 512 h-dims per core
HTL = HL // 128            # 4 local h tiles
KT = M // 128              # 8 contraction tiles for GEMM1
NW = E * HTL               # 32 resident weight tiles per slab

_PROGRAMS = {}
PROFILE = False
LAST_RESULT = None


def _schedule(counts):
    """Chunk list [(expert, padded_width, exact_width, row_offset)].

    Each expert's rows are padded to a multiple of 128 (GEMM2 output-tile
    granularity); GEMM1 only computes the exact_width real columns — the
    trailing pad rows of y hold garbage that the host never gathers.
    """
    ce = [-(-int(c) // 128) * 128 if c else 0 for c in counts]
    chunks = []
    r0 = 0
    for e, c in enumerate(ce):
        done = 0
        while done < c:
            w = min(512, c - done)
            wx = max(1, min(w, int(counts[e]) - done))
            chunks.append((e, w, wx, r0))
            r0 += w
            done += w
    return chunks, r0, ce


def _build_program(counts):
    import concourse.mybir as mybir
    import concourse.tile as tile
    from concourse import bacc

    bf16 = mybir.dt.bfloat16
    f16 = mybir.dt.float16
    f32 = mybir.dt.float32

    chunks, R, ce = _schedule(counts)

    nc = bacc.Bacc("TRN2", target_bir_lowering=False, debug=False,
                   num_devices=N_CORES)

    # dispT[p, k, r] = disp[r, k*128 + p] — partition-major so one 3D DMA
    # fetches a whole chunk's 8 contraction tiles in a single issue
    dispT = nc.declare_dram_parameter("dispT", [128, KT, R], bf16,
                                      isOutput=False)
    # w1[e*HTL+ht, p, k*128+f] = fc1_w[e][core_h0 + ht*128+f, k*128+p]
    w1 = nc.declare_dram_parameter("w1", [NW, 128, M], bf16, isOutput=False)
    # w2[e*HTL+ht, p, n] = fc2_w[e][core_h0 + ht*128+p, n]
    w2 = nc.declare_dram_parameter("w2", [NW, 128, M], bf16, isOutput=False)
    # b1[p, e*HTL+ht] = fc1_b[e][core_h0 + ht*128+p]
    b1 = nc.declare_dram_parameter("b1", [128, NW], f32, isOutput=False)
    # y[p, g, col] = y_row[g*128 + p, col] — partition-major so one 3D DMA
    # writes a whole (chunk, nch) quarter in a single issue
    y = nc.declare_dram_parameter("y", [128, R // 128, M], f16, isOutput=True)

    with tile.TileContext(nc) as tc:
        with (
            tc.tile_pool(name="wres", bufs=1) as wpool,
            tc.tile_pool(name="consts", bufs=1) as cpool,
            tc.tile_pool(name="dt", bufs=3) as dtpool,
            tc.tile_pool(name="ht", bufs=2) as htpool,
            tc.tile_pool(name="ystage", bufs=4) as ypool,
            tc.tile_pool(name="ph", bufs=3, space="PSUM") as phpool,
            tc.tile_pool(name="py", bufs=4, space="PSUM") as pypool,
        ):
            b1_sb = cpool.tile([128, NW], f32, tag="b1")

            # HAM warmup: the PE clock-gate defaults to 1.2 GHz and needs
            # ~3.4us of sustained activity to unthrottle. These dummy matmuls
            # run during the initial input-DMA wait so the first real matmuls
            # start at 2.4 GHz.
            wu = cpool.tile([128, 256], bf16, tag="wu")
            nc.vector.memset(wu[:], 0.0)
            with tc.tile_pool(name="wups", bufs=1, space="PSUM") as wupool:
                wups = wupool.tile([128, 256], f32, tag="wups")
                for _ in range(20):
                    nc.tensor.matmul(wups[:], lhsT=wu[:, :128],
                                     rhs=wu[:, :256], start=True, stop=True)

            # resident weight slabs: 2 x 32 tiles of [128, 1024] bf16.
            w1_sb = [wpool.tile([128, M], bf16, tag=f"w1_{i}",
                                name=f"w1sb_{i}") for i in range(NW)]
            w2_sb = [wpool.tile([128, M], bf16, tag=f"w2_{i}",
                                name=f"w2sb_{i}") for i in range(NW)]

            n_chunks = len(chunks)

            # weight-tile streaming: chunk 0's expert loads up front in
            # first-use order; everything else trickles on the sync queue,
            # rate-limited to smooth HBM demand, with a 4-chunk-lookahead
            # floor so a slab is always fully requested well before its
            # first chunk.
            exp_order = []
            for (e, _, _, _) in chunks:
                if e not in exp_order:
                    exp_order.append(e)
            e_first = exp_order[0]
            nc.sync.dma_start(out=w1_sb[e_first * HTL][:],
                              in_=w1[e_first * HTL])
            wq = []  # remaining weight-tile loads in need order
            for e in exp_order[1:]:
                for ht in range(HTL):
                    wq.append((w1_sb[e * HTL + ht], w1[e * HTL + ht]))
                    wq.append((w2_sb[e * HTL + ht], w2[e * HTL + ht]))
            need_after = {e_first: 0}
            for i, e in enumerate(exp_order[1:]):
                need_after[e] = (i + 1) * 2 * HTL
            wq_done = 0

            def stream_weights(cc):
                nonlocal wq_done
                target = wq_done
                for j in range(cc + 1, min(cc + 5, n_chunks)):
                    target = max(target, need_after[chunks[j][0]])
                target = max(target, min(len(wq), wq_done + 3))
                while wq_done < target:
                    buf, src = wq[wq_done]
                    nc.sync.dma_start(out=buf[:], in_=src)
                    wq_done += 1

            dt_tiles = {}

            def emit_dt(cc):
                e, W, Wx, r0 = chunks[cc]
                t = dtpool.tile([128, KT, 512], bf16, tag="dt",
                                name="dt_sb")
                # 8 separate issues so the transfer fans out across the DMA
                # engines; on sync so the scalar queue stays free for the
                # RELUs (a RELU stuck behind DMA-descriptor generation
                # stalls GEMM1 on psum recycling). Chunk 0 goes on scalar —
                # it is empty until the first RELU, and this keeps the
                # startup-critical dt(0)/w1/dt(1) issues off one queue.
                eng = nc.scalar if cc == 0 else nc.sync
                for k in range(KT):
                    eng.dma_start(out=t[:, k, :Wx],
                                  in_=dispT[:, k, r0:r0 + Wx])
                dt_tiles[cc] = t

            h_tiles = {}

            def emit_g1(cc):
                e, W, Wx, r0 = chunks[cc]
                dt_sb = dt_tiles.pop(cc)
                h_sb = []
                for ht in range(HTL):
                    wt = w1_sb[e * HTL + ht]
                    ph = phpool.tile([128, 512], f32, tag="ph")
                    for k in range(KT):
                        nc.tensor.matmul(
                            ph[:, :Wx],
                            lhsT=wt[:, k * 128:(k + 1) * 128],
                            rhs=dt_sb[:, k, :Wx],
                            start=(k == 0),
                            stop=(k == KT - 1),
                        )
                    hh = htpool.tile([128, 512], bf16, tag=f"h_{ht}")
                    nc.scalar.activation(
                        out=hh[:, :Wx], in_=ph[:, :Wx],
                        func=mybir.ActivationFunctionType.Relu,
                        bias=b1_sb[:, e * HTL + ht:e * HTL + ht + 1],
                        scale=1.0)
                    h_sb.append(hh)
                h_tiles[cc] = h_sb

            def emit_g2(cc):
                e, W, Wx, r0 = chunks[cc]
                g0 = r0 // 128
                ncs = W // 128
                # consolidated y writeback (1 issue per nch) keeps the sync
                # sequencer light; the last chunks use per-csub issues so
                # the final transfer fans out across DMA engines and the
                # kernel tail stays short.
                split_tail = cc >= n_chunks - 2
                h_sb = h_tiles.pop(cc)
                for nch in range(2):
                    ys = ypool.tile([128, 4, 512], f16, tag=f"ys{nch}")
                    for csub in range(ncs):
                        py = pypool.tile([128, 512], f32, tag="py")
                        for ht in range(HTL):
                            nc.tensor.matmul(
                                py[:],
                                lhsT=h_sb[ht][:, csub * 128:(csub + 1) * 128],
                                rhs=w2_sb[e * HTL + ht][:,
                                                        nch * 512:(nch + 1) * 512],
                                start=(ht == 0),
                                stop=(ht == HTL - 1),
                            )
                        nc.vector.tensor_copy(out=ys[:, csub, :], in_=py[:])
                        if split_tail:
                            nc.sync.dma_start(
                                out=y[:, g0 + csub:g0 + csub + 1,
                                      nch * 512:(nch + 1) * 512],
                                in_=ys[:, csub:csub + 1, :])
                    if not split_tail:
                        nc.sync.dma_start(
                            out=y[:, g0:g0 + ncs, nch * 512:(nch + 1) * 512],
                            in_=ys[:, :ncs, :])

            # PE order: G1(0), G1(1), G2(0), G1(2), G2(1), ... — GEMM1 of
            # the next chunk runs between a chunk's GEMM1 and GEMM2, so the
            # trailing activation and the dt prefetch always have a full
            # GEMM1 of cover.
            # prologue issue order tracks first use: w1.ht0, chunk-0 tokens,
            # w1.ht1-3, b1 (first RELU), chunk-1 tokens, then w2 (GEMM2(0))
            emit_dt(0)
            for ht in range(1, HTL):
                nc.sync.dma_start(out=w1_sb[e_first * HTL + ht][:],
                                  in_=w1[e_first * HTL + ht])
            nc.sync.dma_start(out=b1_sb[:], in_=b1[:])
            if n_chunks > 1:
                emit_dt(1)
            for ht in range(HTL):
                nc.sync.dma_start(out=w2_sb[e_first * HTL + ht][:],
                                  in_=w2[e_first * HTL + ht])
            for cc in range(n_chunks):
                if cc + 2 < n_chunks:
                    emit_dt(cc + 2)
                stream_weights(cc)
                emit_g1(cc)
                if cc >= 1:
                    emit_g2(cc - 1)
            emit_g2(n_chunks - 1)

    nc.compile()
    return nc


def _get_program(counts):
    key = tuple(counts)
    if key not in _PROGRAMS:
        _PROGRAMS[key] = _build_program(counts)
    return _PROGRAMS[key]


def _route(x, gate_w):
    """Exact GShard/Tutel k-major top-2 routing in numpy fp32."""
    logits = x @ gate_w  # [T, E]
    m = logits.max(axis=-1, keepdims=True)
    ex = np.exp(logits - m)
    gates = ex / ex.sum(axis=-1, keepdims=True)

    n = x.shape[0]
    ar = np.arange(n)
    e0 = np.argmax(gates, axis=-1)
    g0 = gates[ar, e0]
    gm = gates.copy()
    gm[ar, e0] = -np.inf
    e1 = np.argmax(gm, axis=-1)
    g1 = gates[ar, e1]
    s = g0 + g1
    g0, g1 = g0 / s, g1 / s

    e_flat = np.concatenate([e0, e1])  # k-major
    kt = e_flat.shape[0]
    sort_idx = np.argsort(e_flat, kind="stable")
    sorted_e = e_flat[sort_idx]
    first = np.r_[0, np.flatnonzero(np.diff(sorted_e)) + 1]
    counts = np.diff(np.r_[first, kt])
    grp_start = np.repeat(first, counts)
    pos = np.empty(kt, np.int64)
    pos[sort_idx] = np.arange(kt) - grp_start
    valid = pos < CAP
    slot = np.where(valid, e_flat * CAP + pos, 0)
    return e_flat, valid, slot, np.stack([g0, g1]), np.stack([e0, e1])


def kernel(x, gate_w, fc1_w, fc1_b, fc2_w, fc2_b):
    global LAST_RESULT
    from concourse.bass_utils import run_bass_kernel_spmd

    x = np.asarray(x, np.float32)
    gate_w = np.asarray(gate_w, np.float32)
    fc1_w = np.asarray(fc1_w, np.float32)
    fc1_b = np.asarray(fc1_b, np.float32)
    fc2_w = np.asarray(fc2_w, np.float32)
    fc2_b = np.asarray(fc2_b, np.float32)

    e_flat, valid, slot, g, top_e = _route(x, gate_w)
    pos = slot - e_flat * CAP  # position within expert (valid entries)

    # per-expert valid-row counts, padded to 128 (PE output-tile granularity)
    counts = [int(c) for c in np.bincount(e_flat[valid], minlength=E)]
    ce = [-(-c // 128) * 128 if c else 0 for c in counts]
    seg_off = np.concatenate([[0], np.cumsum(ce)]).astype(np.int64)
    R = int(seg_off[-1])

    # dispatch: pack valid rows by (expert, slot) into [R, M]
    disp = np.zeros((R, M), np.float32)
    tok = np.tile(np.arange(T), K)
    ef_v, pos_v = e_flat[valid], pos[valid]
    disp[seg_off[ef_v] + pos_v] = x[tok[valid]]

    bf = ml_dtypes.bfloat16
    # [p, k, r] = disp[r, k*128+p]
    dispT = np.ascontiguousarray(
        disp.reshape(R, KT, 128).transpose(2, 1, 0)).astype(bf)

    in_maps = []
    for c in range(N_CORES):
        hsl = slice(c * HL, (c + 1) * HL)
        # [e, ht][p, k*128+f] = fc1_w[e, hsl][ht*128+f, k*128+p]
        w1_c = np.ascontiguousarray(
            fc1_w[:, hsl, :].reshape(E, HTL, 128, KT, 128)
            .transpose(0, 1, 4, 3, 2)
        ).reshape(NW, 128, M).astype(bf)
        w2_c = fc2_w[:, hsl, :].reshape(NW, 128, M).astype(bf)
        b1_c = np.ascontiguousarray(
            fc1_b[:, hsl].reshape(NW, 128).T).astype(np.float32)
        in_maps.append({"dispT": dispT, "w1": w1_c, "w2": w2_c, "b1": b1_c})

    nc = _get_program(counts)
    res = run_bass_kernel_spmd(nc, in_maps, core_ids=list(range(N_CORES)),
                               trace=PROFILE)
    LAST_RESULT = res

    y3 = np.zeros((128, R // 128, M), np.float32)
    for c in range(N_CORES):
        y3 += res.results[c]["y"].astype(np.float32)
    y_full = np.ascontiguousarray(y3.transpose(1, 0, 2)).reshape(R, M)

    # combine: weighted gather + fc2 bias contribution
    validK = valid.reshape(K, T)
    eK = e_flat.reshape(K, T)
    posK = np.where(valid, pos, 0).reshape(K, T)
    gv = (g * validK).astype(np.float32)
    out = np.zeros((T, M), np.float32)
    for k in range(K):
        idx = seg_off[eK[k]] + posK[k]
        contrib = y_full[idx] * gv[k][:, None]
        out += np.where(validK[k][:, None], contrib, 0.0)
        out += gv[k][:, None] * fc2_b[top_e[k]]
    return out
